# revision 12
# baseline (speedup 1.0000x reference)
"""Trainium2 Bass kernel for nn_DYS_opt_net: MLP + 51 DYS steps, data-parallel
over the batch on 8 cores.

Math (equivalent to the SVD-projector reference):
  P = pinv(A) A = V V^T with V = R^T, R = L^{-1} A, L = chol(A A^T)  (host, fp64)
  c = pinv(A) b = R^T (L^{-1} b)
  w = relu(d W1 + b1) W2 + b2;  aw = alpha*w,  rc = aw - c
  step:  u = |z| - aw          (2 relu(z) - z = |z|)
         s = relu(z) - rc
         z' = s - (u V) V^T
  output = relu(z_51)

Per core (batch slice of 32 rows, N2-major layout, all-fp16 matmuls):
  mm2 (z2 = (T V^T)^T): m-outer, 8 fp16 matmuls per m-chunk; drains of
  8-chunk groups overlap later m-chunks (DVE: z'=s-z2; Act: |z'|, relu;
  DVE: u16, s).
  mm1 (T^T = V^T u^T): phase A consumes u16 chunks 0..15 (q-outer),
  phase B chunks 16..31 with per-q tt16 casts overlapping the next q.
  Weight-load bound: 512 LDWEIGHTS+MATMUL pairs/iter @ ~26.6 ns.
"""

import os
import sys
from contextlib import ExitStack

import numpy as np

try:
    import concourse.bass as bass
except ImportError:
    sys.path.insert(0, "/opt/trn_rl_repo")
    import concourse.bass as bass

import concourse.tile as tile
from concourse import bacc, mybir
from concourse.bass_utils import run_bass_kernel_spmd

F16 = mybir.dt.float16
F32 = mybir.dt.float32
NP_F16 = np.float16

ALPHA = np.float32(0.05)
NCORES = 8
B, D, H, N1, N2 = 256, 512, 2048, 1024, 4096
BS = B // NCORES          # 32 batch rows per core
TRIPS = int(os.environ.get("DYS_TRIPS", "50"))  # HW-loop trips; +1 prologue mm1, +1 epilogue mm2
UNROLL = os.environ.get("DYS_UNROLL", "0") == "1"
KD, KH, KN2, KN1 = D // 128, H // 128, N2 // 128, N1 // 128   # 4, 16, 32, 8
GD = 8                    # n2-chunks per drain group
NG = KN2 // GD            # 4 drain groups


def _dt_np(x, dt):
    return np.ascontiguousarray(x, dtype=dt)


def _build_program():
    nc = bacc.Bacc("TRN2", target_bir_lowering=False, debug=False,
                   num_devices=NCORES)

    dram = {}
    def din(name, shape, dt):
        dram[name] = nc.dram_tensor(name, list(shape), dt, kind="ExternalInput").ap()
    din("w1t", (128, KH, KD, 128), F16)     # W1 d-major, m-major tiles
    din("dt16", (128, KD, BS), F16)         # d^T slice
    din("w2t", (128, KN2, KH, 128), F16)    # W2 h-major, m-major tiles
    # v16/vthi split so early chunks land before the full 8MB finishes
    for p in range(4):
        din(f"v16_{p}", (128, KN2 // 4, N1), F16)    # V n2-major, k'-split
        din(f"vthi_{p}", (128, KN1, N2 // 4), F16)   # V^T rank-major, m-split
    din("z0t", (128, KN2, BS), F32)         # z0^T slice
    din("cvec", (128, KN2), F32)            # c
    din("ab2", (128, KN2), F32)             # alpha*b2
    din("b1t", (128, KH), F32)              # b1
    out_d = nc.dram_tensor("outt", [128, KN2, BS], F32, kind="ExternalOutput").ap()

    with tile.TileContext(nc) as tc:
        with ExitStack() as ctx:
            res = ctx.enter_context(tc.tile_pool(name="resident", bufs=1))
            st = ctx.enter_context(tc.tile_pool(name="state", bufs=1))
            wstream = ctx.enter_context(tc.tile_pool(name="wstream", bufs=3))
            ps = ctx.enter_context(tc.tile_pool(name="ps", bufs=1, space="PSUM"))

            # ---- resident projector factors (4 separate tiles each so early
            # chunks land before the full 8MB finishes; whole-tile DMAs only) ----
            v16p = []
            vthip = []
            for p in range(4):
                v = res.tile([128, KN2 // 4, N1], F16, name=f"v16_{p}")
                nc.sync.dma_start(v[:], dram[f"v16_{p}"][:])
                v16p.append(v)
            for p in range(4):
                v = res.tile([128, KN1, N2 // 4], F16, name=f"vthi_{p}")
                nc.sync.dma_start(v[:], dram[f"vthi_{p}"][:])
                vthip.append(v)

            # ---- state: one tile per drain group so hazards (tracked at
            # tile granularity) never serialize the PE against drains ----
            ztg = [st.tile([128, GD, BS], F32, name=f"zt{g}") for g in range(NG)]
            u16g = [st.tile([128, GD, BS], F16, name=f"u16{g}") for g in range(NG)]
            sg = [st.tile([128, GD, BS], F32, name=f"s{g}") for g in range(NG)]
            xg = [st.tile([128, GD, BS], F32, name=f"x{g}") for g in range(NG)]
            tg = [st.tile([128, GD, BS], F32, name=f"t{g}") for g in range(NG)]
            aw = st.tile([128, KN2, BS], F32)      # alpha*w (+alpha*b2)
            rc = st.tile([128, KN2, BS], F32)      # aw - c
            tt16 = st.tile([128, KN1, BS], F16)
            cv = st.tile([128, KN2], F32)
            ab2 = st.tile([128, KN2], F32)
            z0t = st.tile([128, KN2, BS], F32)
            nc.sync.dma_start(z0t[:], dram["z0t"][:])
            nc.sync.dma_start(cv[:], dram["cvec"][:])
            nc.sync.dma_start(ab2[:], dram["ab2"][:])

            # ---- MLP: w^T = W2^T relu(W1^T d^T + b1) + b2; aw, rc ----
            dt16 = st.tile([128, KD, BS], F16)
            b1t = st.tile([128, KH], F32)
            nc.sync.dma_start(dt16[:], dram["dt16"][:])
            nc.sync.dma_start(b1t[:], dram["b1t"][:])

            ht_ps = ps.tile([128, KH, BS], F32)
            for m in range(KH):
                w1tile = wstream.tile([128, KD, 128], F16, tag="w1t")
                nc.sync.dma_start(w1tile[:], dram["w1t"][:, m])
                for k in range(KD):
                    nc.tensor.matmul(ht_ps[:, m, :], w1tile[:, k, :],
                                     dt16[:, k, :], start=(k == 0), stop=(k == KD - 1))
            hadd = st.tile([128, KH, BS], F32)
            nc.vector.tensor_add(hadd[:], ht_ps[:],
                                 b1t[:, :, None].to_broadcast((128, KH, BS)))
            ht16 = st.tile([128, KH, BS], F16)
            nc.scalar.activation(ht16[:], hadd[:], mybir.ActivationFunctionType.Relu)

            wt_ps = ps.tile([128, KN2, BS], F32)
            for m in range(KN2):
                w2tile = wstream.tile([128, KH, 128], F16, tag="w2t")
                nc.sync.dma_start(w2tile[:], dram["w2t"][:, m])
                for k in range(KH):
                    nc.tensor.matmul(wt_ps[:, m, :], w2tile[:, k, :], ht16[:, k, :],
                                     start=(k == 0), stop=(k == KH - 1))
            nc.vector.tensor_scalar_mul(aw[:], wt_ps[:], float(ALPHA))
            nc.vector.tensor_add(aw[:], aw[:],
                                 ab2[:, :, None].to_broadcast((128, KN2, BS)))
            nc.vector.tensor_sub(rc[:], aw[:],
                                 cv[:, :, None].to_broadcast((128, KN2, BS)))

            # ---- PSUM for the loop: ping-pong so a drain/cast read never
            # blocks the next chain's writes ----
            ttp = [ps.tile([128, KN1 // 2, BS], F32, name=f"ttp{i}") for i in range(2)]
            z2p = [ps.tile([128, GD, BS], F32, name=f"z2p{i}") for i in range(2)]

            def prep(g):
                """From fresh z' in ztg[g]: u16 and s for the next step."""
                gsl = slice(g * GD, (g + 1) * GD)
                nc.scalar.activation(tg[g][:], ztg[g][:],
                                     mybir.ActivationFunctionType.Abs)
                nc.vector.tensor_sub(u16g[g][:], tg[g][:], aw[:, gsl])
                nc.scalar.activation(xg[g][:], ztg[g][:],
                                     mybir.ActivationFunctionType.Relu)
                nc.vector.tensor_sub(sg[g][:], xg[g][:], rc[:, gsl])

            def mm1_and_cast():
                """tt16 = f16(V^T u^T): one full accumulation chain per q
                (never two open chains in a bank); per-q casts overlap the
                next q's matmuls."""
                for q in range(KN1):
                    for k in range(KN2):
                        nc.tensor.matmul(ttp[q % 2][:, q // 2, :],
                                         v16p[k // 8][:, k % 8, q * 128:(q + 1) * 128],
                                         u16g[k // GD][:, k % GD, :],
                                         start=(k == 0), stop=(k == KN2 - 1))
                    nc.scalar.activation(tt16[:, q, :], ttp[q % 2][:, q // 2, :],
                                         mybir.ActivationFunctionType.Copy)

            def mm2_and_drain(last):
                """z2 = (T V^T)^T per m-chunk; drain groups of GD chunks into
                z' (and u16/s for the next step unless last)."""
                for m in range(KN2):
                    for k in range(KN1):
                        nc.tensor.matmul(z2p[(m // GD) % 2][:, m % GD, :],
                                         vthip[m // 8][:, k, (m % 8) * 128:(m % 8 + 1) * 128],
                                         tt16[:, k, :],
                                         start=(k == 0), stop=(k == KN1 - 1))
                    if m % GD == GD - 1:
                        g = m // GD
                        nc.vector.tensor_sub(ztg[g][:], sg[g][:], z2p[g % 2][:])
                        if last:
                            nc.scalar.activation(xg[g][:], ztg[g][:],
                                                 mybir.ActivationFunctionType.Relu)
                        else:
                            prep(g)

            # ---- prologue: prep from z0, first mm1 ----
            for g in range(NG):
                gsl = slice(g * GD, (g + 1) * GD)
                nc.scalar.activation(tg[g][:], z0t[:, gsl],
                                     mybir.ActivationFunctionType.Abs)
                nc.vector.tensor_sub(u16g[g][:], tg[g][:], aw[:, gsl])
                nc.scalar.activation(xg[g][:], z0t[:, gsl],
                                     mybir.ActivationFunctionType.Relu)
                nc.vector.tensor_sub(sg[g][:], xg[g][:], rc[:, gsl])
            mm1_and_cast()

            # ---- TRIPS full trips + epilogue mm2 ----
            if UNROLL:
                for _ in range(TRIPS):
                    mm2_and_drain(last=False)
                    mm1_and_cast()
            elif TRIPS > 0:
                with tc.For_i(0, TRIPS, 1, hint_engines=(mybir.EngineType.PE,)) as _i:
                    mm2_and_drain(last=False)
                    mm1_and_cast()
            mm2_and_drain(last=True)

            for g in range(NG):
                nc.sync.dma_start(out_d[:, g * GD:(g + 1) * GD, :], xg[g][:])

    nc.compile()
    return nc


_CACHE = {}


def _host_factors(A, b_vec):
    A64 = A.astype(np.float64)
    L = np.linalg.cholesky(A64 @ A64.T)
    R = np.linalg.solve(L, A64)                     # (N1, N2), orthonormal rows
    q = np.linalg.solve(L, b_vec.astype(np.float64))
    c = (R.T @ q).astype(np.float32)                # (N2,)
    VT = R.astype(np.float32)                       # (N1, N2) = V^T
    V = np.ascontiguousarray(VT.T)                  # (N2, N1)
    return V, VT, c


def host_in_maps(d, A, b_vec, W1, b1, W2, b2, z0):
    V, VT, c = _host_factors(A, b_vec)

    v16 = _dt_np(V.reshape(KN2, 128, N1).transpose(1, 0, 2), NP_F16)
    vthi = _dt_np(VT.astype(NP_F16).reshape(KN1, 128, N2).transpose(1, 0, 2), NP_F16)
    w1t = _dt_np(
        W1.astype(NP_F16).reshape(KD, 128, KH, 128).transpose(1, 2, 0, 3), NP_F16)
    w2t = _dt_np(
        W2.astype(NP_F16).reshape(KH, 128, KN2, 128).transpose(1, 2, 0, 3), NP_F16)
    cvec = _dt_np(c.reshape(KN2, 128).T, np.float32)
    ab2 = _dt_np((ALPHA * b2.astype(np.float32)).reshape(KN2, 128).T, np.float32)
    b1t = _dt_np(b1.astype(np.float32).reshape(KH, 128).T, np.float32)

    shared = {"w1t": w1t, "w2t": w2t, "cvec": cvec, "ab2": ab2, "b1t": b1t}
    for p in range(4):
        shared[f"v16_{p}"] = _dt_np(v16[:, p * (KN2 // 4):(p + 1) * (KN2 // 4), :],
                                    NP_F16)
        shared[f"vthi_{p}"] = _dt_np(vthi[:, :, p * (N2 // 4):(p + 1) * (N2 // 4)],
                                     NP_F16)

    in_maps = []
    for i in range(NCORES):
        rows = slice(i * BS, (i + 1) * BS)
        dT = np.ascontiguousarray(d[rows].T)        # (D, BS)
        dt16 = _dt_np(dT.reshape(KD, 128, BS).transpose(1, 0, 2), NP_F16)
        z0T = np.ascontiguousarray(z0[rows].T)      # (N2, BS)
        z0t = _dt_np(z0T.reshape(KN2, 128, BS).transpose(1, 0, 2), np.float32)
        in_maps.append({**shared, "dt16": dt16, "z0t": z0t})
    return in_maps


def kernel(d, A, b_vec, W1, b1, W2, b2, z0):
    in_maps = host_in_maps(d, A, b_vec, W1, b1, W2, b2, z0)

    if "nc" not in _CACHE:
        _CACHE["nc"] = _build_program()
    nc = _CACHE["nc"]

    trace = os.environ.get("DYS_TRACE", "0") == "1"
    res = run_bass_kernel_spmd(nc, in_maps, list(range(NCORES)), trace=trace)
    _CACHE["last_result"] = res

    out = np.empty((B, N2), dtype=np.float32)
    for i in range(NCORES):
        arr = res.results[i]["outt"]                # (128, KN2, BS)
        out[i * BS:(i + 1) * BS] = arr.transpose(2, 1, 0).reshape(BS, N2)
    return out
